# revision 6
# baseline (speedup 1.0000x reference)
"""Trainium2 Bass kernel for a double-path sign-quantized (ITQ) linear layer.

  y = ((x @ (sign(V).T * v2f)) * (v1*u2)) @ (sign(U).T scaled by u1)
      + same for _R path
      + bias

Sharding: data-parallel over tokens across 8 NeuronCores (8192 tokens -> 1024
per core). Weights are pre-quantized on host (sign() + scale folds + transpose
to contraction-major layout + bf16 cast); each core runs the full two-matmul
chain for its token slice. No collectives.

Device-side dataflow per core:
  phase 1: DMA-cast x slice f32->bf16, PE-transpose to xT [in, tok],
           mm1: hT[split, tok] = VT.T @ xT accumulated over 32 k-tiles,
           PSUM->SBUF copy applies (v1*u2) per-partition scale + bf16 cast.
  phase 2: mm2: y[tok, out] accumulates 16 (path, split-tile) rounds in PSUM,
           DVE adds broadcast bias, DMA out f32.
"""

import os
import sys

for _p in ("/opt/trn_rl_repo", "/root/.axon_site/_ro/trn_rl_repo"):
    if os.path.isdir(_p) and _p not in sys.path:
        sys.path.insert(0, _p)

import numpy as np
import ml_dtypes

import concourse.bass as bass
import concourse.mybir as mybir
import concourse.tile as tile
from concourse import bacc, bass_utils
from concourse.masks import make_identity

P = 128
IN_F, OUT_F, SPLIT = 4096, 4096, 1024
B, S = 2, 4096
N_CORES = 8
TOK = (B * S) // N_CORES        # 1024 tokens per core
KI = IN_F // P                  # 32 k-tiles for mm1
ST = SPLIT // P                 # 8 split tiles per path
TT = TOK // P                   # 8 token tiles
OQ = 2                          # out-feature quads of 2048
NP = 2                          # paths (main, residual)

F32 = mybir.dt.float32
BF16 = mybir.dt.bfloat16

_CACHE = {}
last_exec_time_ns = None
last_results = None


def _build():
    nc = bacc.Bacc("TRN2", target_bir_lowering=False, debug=False,
                   num_devices=N_CORES)

    xs = nc.dram_tensor("xs", [TOK, IN_F], F32, kind="ExternalInput")
    # vt rows: (path*8+st)*128+p ; cols: k*128+j  == sign(V[st*128+j, k*128+p])*v2[k*128+p]
    vt = nc.dram_tensor("vt", [NP * ST * P, KI * P], BF16, kind="ExternalInput")
    # ut rows: ((path*2+oq)*8+k)*128+p ; cols j in [0,2048) == sign(U[oq*2048+j, k*128+p])*u1[oq*2048+j]
    ut = nc.dram_tensor("ut", [NP * OQ * ST * P, 2048], BF16, kind="ExternalInput")
    sc = nc.dram_tensor("sc", [P, NP * ST], F32, kind="ExternalInput")
    bb = nc.dram_tensor("bb", [P, OUT_F], F32, kind="ExternalInput")
    y = nc.dram_tensor("y", [TOK, OUT_F], F32, kind="ExternalOutput")

    xs_ap, vt_ap, ut_ap, sc_ap, bb_ap, y_ap = (
        t.ap() for t in (xs, vt, ut, sc, bb, y))

    with tile.TileContext(nc) as tc:
        with tc.tile_pool(name="const", bufs=1) as const, \
             tc.tile_pool(name="ht", bufs=1) as ht_pool:
            ident = const.tile([P, P], BF16)
            make_identity(nc, ident[:])
            sc_sb = const.tile([P, NP * ST], F32)
            nc.sync.dma_start(out=sc_sb[:], in_=sc_ap[:, :])
            # hT[p, (path*8+st)*1024 + t] = scaled h.T, bf16
            hT = ht_pool.tile([P, NP * SPLIT * TT], BF16)

            # ---------------- phase 1: x transpose + mm1 ----------------
            with tc.tile_pool(name="xc", bufs=2) as xc_pool, \
                 tc.tile_pool(name="xT", bufs=1) as xT_pool, \
                 tc.tile_pool(name="vtst", bufs=3) as vt_pool, \
                 tc.tile_pool(name="tpsum", bufs=2, space="PSUM") as tpsum, \
                 tc.tile_pool(name="ps1", bufs=4, space="PSUM") as ps1:
                xT = xT_pool.tile([P, KI * TOK], BF16)  # [in-part, k*TOK + t]
                for tt in range(TT):
                    xc = xc_pool.tile([P, IN_F], BF16, tag="xc")
                    # SWDGE dma with f32->bf16 cast
                    nc.gpsimd.dma_start(
                        out=xc[:], in_=xs_ap[tt * P:(tt + 1) * P, :])
                    for g in range(KI // 4):
                        tp = tpsum.tile([P, 512], BF16, tag="tp")
                        for j in range(4):
                            k = g * 4 + j
                            nc.tensor.transpose(
                                tp[:, j * P:(j + 1) * P],
                                xc[:, k * P:(k + 1) * P], ident[:])
                        dst = (xT[:]
                               .rearrange("p (k t) -> p k t", k=KI)
                               [:, g * 4:(g + 1) * 4, tt * P:(tt + 1) * P])
                        src = tp[:].rearrange("p (j t) -> p j t", j=4)
                        nc.scalar.activation(
                            dst, src, mybir.ActivationFunctionType.Copy)

                for path in range(NP):
                    for st in range(ST):
                        r0 = (path * ST + st) * P
                        vtt = vt_pool.tile([P, KI * P], BF16, tag="vtst")
                        nc.sync.dma_start(
                            out=vtt[:], in_=vt_ap[r0:r0 + P, :])
                        psA = ps1.tile([P, 512], F32, tag="ps1")
                        psB = ps1.tile([P, 512], F32, tag="ps1")
                        for k in range(KI):
                            lhsT = vtt[:, k * P:(k + 1) * P]
                            nc.tensor.matmul(
                                psA[:], lhsT, xT[:, k * TOK:k * TOK + 512],
                                start=(k == 0), stop=(k == KI - 1))
                            nc.tensor.matmul(
                                psB[:], lhsT, xT[:, k * TOK + 512:(k + 1) * TOK],
                                start=(k == 0), stop=(k == KI - 1))
                        base = (path * ST + st) * TOK
                        sccol = sc_sb[:, path * ST + st:path * ST + st + 1]
                        nc.scalar.activation(
                            hT[:, base:base + 512], psA[:],
                            mybir.ActivationFunctionType.Copy, scale=sccol)
                        nc.scalar.activation(
                            hT[:, base + 512:base + TOK], psB[:],
                            mybir.ActivationFunctionType.Copy, scale=sccol)

            # ---------------- phase 2: mm2 + bias + store ----------------
            with tc.tile_pool(name="utq", bufs=24) as ut_pool, \
                 tc.tile_pool(name="bbp", bufs=1) as bb_pool, \
                 tc.tile_pool(name="ysb", bufs=3) as y_pool, \
                 tc.tile_pool(name="ps2", bufs=8, space="PSUM") as ps2:
                bb_sb = bb_pool.tile([P, OUT_F], F32)
                nc.sync.dma_start(out=bb_sb[:], in_=bb_ap[:, :])
                for oq in range(OQ):
                    uts = []
                    for path in range(NP):
                        for k in range(ST):
                            r0 = ((path * OQ + oq) * ST + k) * P
                            u = ut_pool.tile([P, 2048], BF16, tag="utq")
                            nc.sync.dma_start(
                                out=u[:], in_=ut_ap[r0:r0 + P, :])
                            uts.append((path, k, u))
                    for tt in range(TT):
                        pss = []
                        for _q in range(4):
                            psq = ps2.tile([P, 512], F32, tag="ps2")
                            pss.append(psq)
                        for i, (path, k, u) in enumerate(uts):
                            col = (path * ST + k) * TOK + tt * P
                            lhsT = hT[:, col:col + P]
                            for q in range(4):
                                nc.tensor.matmul(
                                    pss[q][:], lhsT,
                                    u[:, q * 512:(q + 1) * 512],
                                    start=(i == 0), stop=(i == len(uts) - 1))
                        ysb = y_pool.tile([P, 2048], F32, tag="ysb")
                        for q in range(4):
                            nc.vector.tensor_add(
                                ysb[:, q * 512:(q + 1) * 512], pss[q][:],
                                bb_sb[:, oq * 2048 + q * 512:
                                      oq * 2048 + (q + 1) * 512])
                        nc.sync.dma_start(
                            out=y_ap[tt * P:(tt + 1) * P,
                                     oq * 2048:(oq + 1) * 2048],
                            in_=ysb[:])

    nc.compile()
    return nc


def _prep_host(x, V, U, v2, v1, u2, u1, V_R, U_R, v2_R, v1_R, u2_R, u1_R,
               bias):
    bf = ml_dtypes.bfloat16
    x2 = np.ascontiguousarray(np.asarray(x, dtype=np.float32)
                              .reshape(B * S, IN_F))

    def prep_vt(Vm, v2m):
        # VT[k, s] = sign(V[s, k]) * v2[k]  -> layout [st, p, k*128+j]
        VT = (np.sign(np.asarray(Vm, np.float32)).T
              * np.asarray(v2m, np.float32).reshape(IN_F, 1)).astype(bf)
        # VT [IN_F, SPLIT] -> [KI, P(p), ST, P(j)] -> [ST, P(p), KI, P(j)]
        return (VT.reshape(KI, P, ST, P).transpose(2, 1, 0, 3)
                .reshape(ST * P, KI * P))

    def prep_ut(Um, u1m):
        # UT[s, o] = sign(U[o, s]) * u1[o] -> layout [(oq, k, p), j]
        UT = (np.sign(np.asarray(Um, np.float32)).T
              * np.asarray(u1m, np.float32).reshape(1, OUT_F)).astype(bf)
        # UT [SPLIT, OUT_F] -> [ST, P, OQ, 2048] -> [OQ, ST, P, 2048]
        return (UT.reshape(ST, P, OQ, 2048).transpose(2, 0, 1, 3)
                .reshape(OQ * ST * P, 2048))

    vt_host = np.concatenate([prep_vt(V, v2), prep_vt(V_R, v2_R)], axis=0)
    ut_host = np.concatenate([prep_ut(U, u1), prep_ut(U_R, u1_R)], axis=0)

    sc_host = np.empty((P, NP * ST), np.float32)
    sc_host[:, 0:ST] = (np.asarray(v1, np.float32)
                        * np.asarray(u2, np.float32)).reshape(ST, P).T
    sc_host[:, ST:] = (np.asarray(v1_R, np.float32)
                       * np.asarray(u2_R, np.float32)).reshape(ST, P).T
    bb_host = np.tile(np.asarray(bias, np.float32).reshape(1, OUT_F), (P, 1))
    return x2, vt_host, ut_host, sc_host, bb_host


def kernel(x, V, U, v2, v1, u2, u1, V_R, U_R, v2_R, v1_R, u2_R, u1_R, bias):
    global last_exec_time_ns, last_results
    if "nc" not in _CACHE:
        _CACHE["nc"] = _build()
    nc = _CACHE["nc"]

    x2, vt_host, ut_host, sc_host, bb_host = _prep_host(
        x, V, U, v2, v1, u2, u1, V_R, U_R, v2_R, v1_R, u2_R, u1_R, bias)

    in_maps = []
    for c in range(N_CORES):
        in_maps.append({
            "xs": np.ascontiguousarray(x2[c * TOK:(c + 1) * TOK]),
            "vt": vt_host,
            "ut": ut_host,
            "sc": sc_host,
            "bb": bb_host,
        })

    res = bass_utils.run_bass_kernel_spmd(
        nc, in_maps, core_ids=list(range(N_CORES)), trace=False)
    last_results = res
    out = np.concatenate([r["y"] for r in res.results], axis=0)
    return out.reshape(B, S, OUT_F).astype(np.float32)


def time_kernel(iters=8, **inputs):
    """Time device execution: inputs pre-placed on device, min wall over iters."""
    import time as _time
    import jax
    from jax.sharding import Mesh, PartitionSpec, NamedSharding
    from jax.experimental.shard_map import shard_map
    from concourse import bass2jax

    if "nc" not in _CACHE:
        _CACHE["nc"] = _build()
    nc = _CACHE["nc"]
    x2, vt_host, ut_host, sc_host, bb_host = _prep_host(**inputs)
    host = {"xs": x2.reshape(N_CORES, TOK, IN_F),
            "vt": vt_host, "ut": ut_host, "sc": sc_host, "bb": bb_host}

    bass2jax.install_neuronx_cc_hook()
    partition_name = (nc.partition_id_tensor.name
                      if nc.partition_id_tensor else None)
    in_names, out_names, out_avals, zero_outs = [], [], [], []
    for alloc in nc.m.functions[0].allocations:
        if not isinstance(alloc, mybir.MemoryLocationSet):
            continue
        name = alloc.memorylocations[0].name
        if alloc.kind == "ExternalInput":
            if name != partition_name:
                in_names.append(name)
        elif alloc.kind == "ExternalOutput":
            out_names.append(name)
            shape = tuple(alloc.tensor_shape)
            dtype = mybir.dt.np(alloc.dtype)
            out_avals.append(jax.core.ShapedArray(shape, dtype))
            zero_outs.append(np.zeros((N_CORES * shape[0], *shape[1:]), dtype))
    n_params = len(in_names)
    all_names = in_names + out_names
    if partition_name is not None:
        all_names = all_names + [partition_name]

    def _body(*args):
        operands = list(args)
        if partition_name is not None:
            operands.append(bass2jax.partition_id_tensor())
        outs = bass2jax._bass_exec_p.bind(
            *operands, out_avals=tuple(out_avals), in_names=tuple(all_names),
            out_names=tuple(out_names), lowering_input_output_aliases=(),
            sim_require_finite=True, sim_require_nnan=True, nc=nc)
        return tuple(outs)

    devices = jax.devices()[:N_CORES]
    mesh = Mesh(np.asarray(devices), ("core",))
    spec = NamedSharding(mesh, PartitionSpec("core"))
    donate = tuple(range(n_params, n_params + len(out_names)))
    sharded = jax.jit(
        shard_map(_body, mesh=mesh,
                  in_specs=(PartitionSpec("core"),) * (n_params + len(out_names)),
                  out_specs=(PartitionSpec("core"),) * len(out_names)),
        donate_argnums=donate, keep_unused=True)

    concat_in = []
    for name in in_names:
        h = host[name]
        if name == "xs":
            concat_in.append(np.ascontiguousarray(h.reshape(-1, IN_F)))
        else:
            concat_in.append(np.concatenate([h] * N_CORES, axis=0))
    dev_in = [jax.device_put(a, spec) for a in concat_in]
    jax.block_until_ready(dev_in)

    times = []
    out = None
    for _ in range(iters):
        dev_zero = [jax.device_put(z, spec) for z in zero_outs]
        jax.block_until_ready(dev_zero)
        t0 = _time.perf_counter()
        out = sharded(*dev_in, *dev_zero)
        jax.block_until_ready(out)
        times.append(_time.perf_counter() - t0)
    y = np.asarray(out[0]).reshape(B, S, OUT_F)
    return times, y


# revision 11
# speedup vs baseline: 1.4228x; 1.4228x over previous
"""Trainium2 Bass kernel for a double-path sign-quantized (ITQ) linear layer.

  y = ((x @ (sign(V).T * v2f)) * (v1*u2)) @ (sign(U).T scaled by u1)
      + same for _R path
      + bias

Sharding: data-parallel over tokens across 8 NeuronCores (8192 tokens -> 1024
per core). Weights are pre-quantized on host (sign() + scale folds + transpose
to contraction-major layout + bf16 cast); each core runs the full two-matmul
chain for its token slice. No collectives.

Device-side dataflow per core:
  phase 1: DMA-cast x slice f32->bf16, PE-transpose to xT [in, tok],
           mm1: hT[split, tok] = VT.T @ xT accumulated over 32 k-tiles,
           PSUM->SBUF copy applies (v1*u2) per-partition scale + bf16 cast.
  phase 2: mm2: y[tok, out] accumulates 16 (path, split-tile) rounds in PSUM,
           DVE adds broadcast bias, DMA out f32.
"""

import os
import sys

for _p in ("/opt/trn_rl_repo", "/root/.axon_site/_ro/trn_rl_repo"):
    if os.path.isdir(_p) and _p not in sys.path:
        sys.path.insert(0, _p)

import numpy as np
import ml_dtypes

import concourse.bass as bass
import concourse.mybir as mybir
import concourse.tile as tile
from concourse import bacc, bass_utils
from concourse.masks import make_identity

P = 128
IN_F, OUT_F, SPLIT = 4096, 4096, 1024
B, S = 2, 4096
N_CORES = 8
TOK = (B * S) // N_CORES        # 1024 tokens per core
KI = IN_F // P                  # 32 k-tiles for mm1
ST = SPLIT // P                 # 8 split tiles per path
TT = TOK // P                   # 8 token tiles
OQ = 2                          # out-feature quads of 2048
NP = 2                          # paths (main, residual)

F32 = mybir.dt.float32
BF16 = mybir.dt.bfloat16

_CACHE = {}
last_exec_time_ns = None
last_results = None


def _build(reps=1):
    nc = bacc.Bacc("TRN2", target_bir_lowering=False, debug=False,
                   num_devices=N_CORES)

    xb = nc.dram_tensor("xb", [TOK, IN_F], BF16, kind="ExternalInput")
    # vt rows: (path*8+st)*128+p ; cols: k*128+j  == sign(V[st*128+j, k*128+p])*v2[k*128+p]
    vt = nc.dram_tensor("vt", [NP * ST * P, KI * P], BF16, kind="ExternalInput")
    # ut rows: ((path*2+oq)*8+k)*128+p ; cols j == sign(U[oq*2048+j, k*128+p])*u1[oq*2048+j]
    ut = nc.dram_tensor("ut", [NP * OQ * ST * P, 2048], BF16, kind="ExternalInput")
    sc = nc.dram_tensor("sc", [P, NP * ST], F32, kind="ExternalInput")
    bb = nc.dram_tensor("bb", [P, OUT_F], F32, kind="ExternalInput")
    y = nc.dram_tensor("y", [TOK, OUT_F], F32, kind="ExternalOutput")

    xb_ap, vt_ap, ut_ap, sc_ap, bb_ap, y_ap = (
        t.ap() for t in (xb, vt, ut, sc, bb, y))

    def phase1(nc, tc, ident, sc_sb, hT):
        with tc.tile_pool(name="xT", bufs=1) as xT_pool, \
             tc.tile_pool(name="vtst", bufs=3) as vt_pool, \
             tc.tile_pool(name="ps1", bufs=4, space="PSUM") as ps1:
            # xT split into two half-token tiles so mm1 can start after the
            # first half's xbar transposes land.
            xTa = xT_pool.tile([P, KI * 512], BF16)
            xTb = xT_pool.tile([P, KI * 512], BF16)
            for half, xTh in ((0, xTa), (1, xTb)):
                for k in range(KI):
                    nc.sync.dma_start_transpose(
                        out=xTh[:, k * 512:(k + 1) * 512],
                        in_=xb_ap[half * 512:(half + 1) * 512,
                                  k * P:(k + 1) * P])

            for path in range(NP):
                for st in range(ST):
                    r0 = (path * ST + st) * P
                    vtt = vt_pool.tile([P, KI * P], BF16, tag="vtst")
                    nc.sync.dma_start(
                        out=vtt[:], in_=vt_ap[r0:r0 + P, :])
                    psA = ps1.tile([P, 512], F32, tag="ps1")
                    psB = ps1.tile([P, 512], F32, tag="ps1")
                    for k in range(KI):
                        lhsT = vtt[:, k * P:(k + 1) * P]
                        nc.tensor.matmul(
                            psA[:], lhsT, xTa[:, k * 512:(k + 1) * 512],
                            start=(k == 0), stop=(k == KI - 1))
                        nc.tensor.matmul(
                            psB[:], lhsT, xTb[:, k * 512:(k + 1) * 512],
                            start=(k == 0), stop=(k == KI - 1))
                    base = (path * ST + st) * TOK
                    sccol = sc_sb[:, path * ST + st:path * ST + st + 1]
                    nc.scalar.activation(
                        hT[:, base:base + 512], psA[:],
                        mybir.ActivationFunctionType.Copy, scale=sccol)
                    nc.scalar.activation(
                        hT[:, base + 512:base + TOK], psB[:],
                        mybir.ActivationFunctionType.Copy, scale=sccol)

    def phase2(nc, tc, hT):
        with tc.tile_pool(name="utq", bufs=24) as ut_pool, \
             tc.tile_pool(name="bbp", bufs=1) as bb_pool, \
             tc.tile_pool(name="ysb", bufs=3) as y_pool, \
             tc.tile_pool(name="ps2", bufs=8, space="PSUM") as ps2:
            bb_sb = bb_pool.tile([P, OUT_F], F32)
            nc.sync.dma_start(out=bb_sb[:], in_=bb_ap[:, :])
            for oq in range(OQ):
                uts = []
                for path in range(NP):
                    for k in range(ST):
                        r0 = ((path * OQ + oq) * ST + k) * P
                        u = ut_pool.tile([P, 2048], BF16, tag="utq")
                        nc.sync.dma_start(
                            out=u[:], in_=ut_ap[r0:r0 + P, :])
                        uts.append((path, k, u))
                for tt in range(TT):
                    pss = []
                    for _q in range(4):
                        psq = ps2.tile([P, 512], F32, tag="ps2")
                        pss.append(psq)
                    for i, (path, k, u) in enumerate(uts):
                        col = (path * ST + k) * TOK + tt * P
                        lhsT = hT[:, col:col + P]
                        for q in range(4):
                            nc.tensor.matmul(
                                pss[q][:], lhsT,
                                u[:, q * 512:(q + 1) * 512],
                                start=(i == 0), stop=(i == len(uts) - 1))
                    ysb = y_pool.tile([P, 2048], F32, tag="ysb")
                    for q in range(4):
                        nc.vector.tensor_add(
                            ysb[:, q * 512:(q + 1) * 512], pss[q][:],
                            bb_sb[:, oq * 2048 + q * 512:
                                  oq * 2048 + (q + 1) * 512])
                    nc.sync.dma_start(
                        out=y_ap[tt * P:(tt + 1) * P,
                                 oq * 2048:(oq + 1) * 2048],
                        in_=ysb[:])

    with tile.TileContext(nc) as tc:
        with tc.tile_pool(name="const", bufs=1) as const, \
             tc.tile_pool(name="ht", bufs=1) as ht_pool:
            ident = const.tile([P, P], BF16)
            make_identity(nc, ident[:])
            sc_sb = const.tile([P, NP * ST], F32)
            nc.sync.dma_start(out=sc_sb[:], in_=sc_ap[:, :])
            # hT[p, (path*8+st)*1024 + t] = scaled h.T, bf16
            hT = ht_pool.tile([P, NP * SPLIT * TT], BF16)
            for _rep in range(reps):
                phase1(nc, tc, ident, sc_sb, hT)
                phase2(nc, tc, hT)

    nc.compile()
    return nc


def _prep_host(x, V, U, v2, v1, u2, u1, V_R, U_R, v2_R, v1_R, u2_R, u1_R,
               bias):
    bf = ml_dtypes.bfloat16
    x2 = np.ascontiguousarray(np.asarray(x, dtype=np.float32)
                              .reshape(B * S, IN_F)).astype(bf)

    def prep_vt(Vm, v2m):
        # VT[k, s] = sign(V[s, k]) * v2[k]  -> layout [st, p, k*128+j]
        VT = (np.sign(np.asarray(Vm, np.float32)).T
              * np.asarray(v2m, np.float32).reshape(IN_F, 1)).astype(bf)
        # VT [IN_F, SPLIT] -> [KI, P(p), ST, P(j)] -> [ST, P(p), KI, P(j)]
        return (VT.reshape(KI, P, ST, P).transpose(2, 1, 0, 3)
                .reshape(ST * P, KI * P))

    def prep_ut(Um, u1m):
        # UT[s, o] = sign(U[o, s]) * u1[o] -> layout [(oq, k, p), j]
        UT = (np.sign(np.asarray(Um, np.float32)).T
              * np.asarray(u1m, np.float32).reshape(1, OUT_F)).astype(bf)
        # UT [SPLIT, OUT_F] -> [ST, P, OQ, 2048] -> [OQ, ST, P, 2048]
        return (UT.reshape(ST, P, OQ, 2048).transpose(2, 0, 1, 3)
                .reshape(OQ * ST * P, 2048))

    vt_host = np.concatenate([prep_vt(V, v2), prep_vt(V_R, v2_R)], axis=0)
    ut_host = np.concatenate([prep_ut(U, u1), prep_ut(U_R, u1_R)], axis=0)

    sc_host = np.empty((P, NP * ST), np.float32)
    sc_host[:, 0:ST] = (np.asarray(v1, np.float32)
                        * np.asarray(u2, np.float32)).reshape(ST, P).T
    sc_host[:, ST:] = (np.asarray(v1_R, np.float32)
                       * np.asarray(u2_R, np.float32)).reshape(ST, P).T
    bb_host = np.tile(np.asarray(bias, np.float32).reshape(1, OUT_F), (P, 1))
    return x2, vt_host, ut_host, sc_host, bb_host


def kernel(x, V, U, v2, v1, u2, u1, V_R, U_R, v2_R, v1_R, u2_R, u1_R, bias):
    global last_exec_time_ns, last_results
    if 1 not in _CACHE:
        _CACHE[1] = _build()
    nc = _CACHE[1]

    x2, vt_host, ut_host, sc_host, bb_host = _prep_host(
        x, V, U, v2, v1, u2, u1, V_R, U_R, v2_R, v1_R, u2_R, u1_R, bias)

    in_maps = []
    for c in range(N_CORES):
        in_maps.append({
            "xb": np.ascontiguousarray(x2[c * TOK:(c + 1) * TOK]),
            "vt": vt_host,
            "ut": ut_host,
            "sc": sc_host,
            "bb": bb_host,
        })

    res = bass_utils.run_bass_kernel_spmd(
        nc, in_maps, core_ids=list(range(N_CORES)), trace=False)
    last_results = res
    out = np.concatenate([r["y"] for r in res.results], axis=0)
    return out.reshape(B, S, OUT_F).astype(np.float32)


def time_kernel(iters=8, reps=1, **inputs):
    """Time device execution: inputs pre-placed on device, min wall over iters."""
    import time as _time
    import jax
    from jax.sharding import Mesh, PartitionSpec, NamedSharding
    from jax.experimental.shard_map import shard_map
    from concourse import bass2jax

    if reps not in _CACHE:
        _CACHE[reps] = _build(reps)
    nc = _CACHE[reps]
    x2, vt_host, ut_host, sc_host, bb_host = _prep_host(**inputs)
    host = {"xb": x2.reshape(N_CORES, TOK, IN_F),
            "vt": vt_host, "ut": ut_host, "sc": sc_host, "bb": bb_host}

    bass2jax.install_neuronx_cc_hook()
    partition_name = (nc.partition_id_tensor.name
                      if nc.partition_id_tensor else None)
    in_names, out_names, out_avals, zero_outs = [], [], [], []
    for alloc in nc.m.functions[0].allocations:
        if not isinstance(alloc, mybir.MemoryLocationSet):
            continue
        name = alloc.memorylocations[0].name
        if alloc.kind == "ExternalInput":
            if name != partition_name:
                in_names.append(name)
        elif alloc.kind == "ExternalOutput":
            out_names.append(name)
            shape = tuple(alloc.tensor_shape)
            dtype = mybir.dt.np(alloc.dtype)
            out_avals.append(jax.core.ShapedArray(shape, dtype))
            zero_outs.append(np.zeros((N_CORES * shape[0], *shape[1:]), dtype))
    n_params = len(in_names)
    all_names = in_names + out_names
    if partition_name is not None:
        all_names = all_names + [partition_name]

    def _body(*args):
        operands = list(args)
        if partition_name is not None:
            operands.append(bass2jax.partition_id_tensor())
        outs = bass2jax._bass_exec_p.bind(
            *operands, out_avals=tuple(out_avals), in_names=tuple(all_names),
            out_names=tuple(out_names), lowering_input_output_aliases=(),
            sim_require_finite=True, sim_require_nnan=True, nc=nc)
        return tuple(outs)

    devices = jax.devices()[:N_CORES]
    mesh = Mesh(np.asarray(devices), ("core",))
    spec = NamedSharding(mesh, PartitionSpec("core"))
    donate = tuple(range(n_params, n_params + len(out_names)))
    sharded = jax.jit(
        shard_map(_body, mesh=mesh,
                  in_specs=(PartitionSpec("core"),) * (n_params + len(out_names)),
                  out_specs=(PartitionSpec("core"),) * len(out_names)),
        donate_argnums=donate, keep_unused=True)

    concat_in = []
    for name in in_names:
        h = host[name]
        if name == "xb":
            concat_in.append(np.ascontiguousarray(h.reshape(-1, IN_F)))
        else:
            concat_in.append(np.concatenate([h] * N_CORES, axis=0))
    dev_in = [jax.device_put(a, spec) for a in concat_in]
    jax.block_until_ready(dev_in)

    times = []
    out = None
    for _ in range(iters):
        dev_zero = [jax.device_put(z, spec) for z in zero_outs]
        jax.block_until_ready(dev_zero)
        t0 = _time.perf_counter()
        out = sharded(*dev_in, *dev_zero)
        jax.block_until_ready(out)
        times.append(_time.perf_counter() - t0)
    y = np.asarray(out[0]).reshape(B, S, OUT_F)
    return times, y


# revision 13
# speedup vs baseline: 101.8004x; 71.5494x over previous
"""Trainium2 Bass kernel for a double-path sign-quantized (ITQ) linear layer.

  y = ((x @ (sign(V).T * v2f)) * (v1*u2)) @ (sign(U).T scaled by u1)
      + same for _R path
      + bias

Sharding: data-parallel over tokens across 8 NeuronCores (8192 tokens -> 1024
per core). Weights are pre-quantized on host (sign() + scale folds + transpose
to contraction-major layout + bf16 cast); each core runs the full two-matmul
chain for its token slice. No collectives.

Device-side dataflow per core:
  phase 1: DMA-cast x slice f32->bf16, PE-transpose to xT [in, tok],
           mm1: hT[split, tok] = VT.T @ xT accumulated over 32 k-tiles,
           PSUM->SBUF copy applies (v1*u2) per-partition scale + bf16 cast.
  phase 2: mm2: y[tok, out] accumulates 16 (path, split-tile) rounds in PSUM,
           DVE adds broadcast bias, DMA out f32.
"""

import os
import sys

for _p in ("/opt/trn_rl_repo", "/root/.axon_site/_ro/trn_rl_repo"):
    if os.path.isdir(_p) and _p not in sys.path:
        sys.path.insert(0, _p)

import numpy as np
import ml_dtypes

import concourse.bass as bass
import concourse.mybir as mybir
import concourse.tile as tile
from concourse import bacc, bass_utils
from concourse.masks import make_identity

P = 128
IN_F, OUT_F, SPLIT = 4096, 4096, 1024
B, S = 2, 4096
N_CORES = 8
TOK = (B * S) // N_CORES        # 1024 tokens per core
KI = IN_F // P                  # 32 k-tiles for mm1
ST = SPLIT // P                 # 8 split tiles per path
TT = TOK // P                   # 8 token tiles
OQ = 2                          # out-feature quads of 2048
NP = 2                          # paths (main, residual)

F32 = mybir.dt.float32
BF16 = mybir.dt.bfloat16

_CACHE = {}
last_exec_time_ns = None
last_results = None


def _build(reps=1):
    nc = bacc.Bacc("TRN2", target_bir_lowering=False, debug=False,
                   num_devices=N_CORES)

    xs = nc.dram_tensor("xs", [TOK, IN_F], F32, kind="ExternalInput")
    # vt rows: (path*8+st)*128+p ; cols: k*128+j  == sign(V[st*128+j, k*128+p])*v2[k*128+p]
    vt = nc.dram_tensor("vt", [NP * ST * P, KI * P], BF16, kind="ExternalInput")
    # ut rows: ((path*2+oq)*8+k)*128+p ; cols j == sign(U[oq*2048+j, k*128+p])*u1[oq*2048+j]
    ut = nc.dram_tensor("ut", [NP * OQ * ST * P, 2048], BF16, kind="ExternalInput")
    sc = nc.dram_tensor("sc", [P, NP * ST], F32, kind="ExternalInput")
    bb = nc.dram_tensor("bb", [P, OUT_F], F32, kind="ExternalInput")
    y = nc.dram_tensor("y", [TOK, OUT_F], F32, kind="ExternalOutput")

    xs_ap, vt_ap, ut_ap, sc_ap, bb_ap, y_ap = (
        t.ap() for t in (xs, vt, ut, sc, bb, y))

    def phase1(nc, tc, ident, sc_sb, hT):
        with tc.tile_pool(name="xc", bufs=3) as xc_pool, \
             tc.tile_pool(name="xT", bufs=1) as xT_pool, \
             tc.tile_pool(name="vtst", bufs=3) as vt_pool, \
             tc.tile_pool(name="tpsum", bufs=2, space="PSUM") as tpsum, \
             tc.tile_pool(name="ps1", bufs=6, space="PSUM") as ps1:
            # xT split into token halves: the first half's matmul chains
            # start after only 4 x-tiles are cast+transposed.
            xTa = xT_pool.tile([P, KI * 512], BF16)
            xTb = xT_pool.tile([P, KI * 512], BF16)
            for tt in range(TT):
                xTh = xTa if tt < 4 else xTb
                xc = xc_pool.tile([P, IN_F], BF16, tag="xc")
                # SWDGE dma with f32->bf16 cast
                nc.gpsimd.dma_start(
                    out=xc[:], in_=xs_ap[tt * P:(tt + 1) * P, :])
                for g in range(KI // 4):
                    tp = tpsum.tile([P, 512], BF16, tag="tp")
                    for j in range(4):
                        k = g * 4 + j
                        nc.tensor.transpose(
                            tp[:, j * P:(j + 1) * P],
                            xc[:, k * P:(k + 1) * P], ident[:])
                    dst = (xTh[:]
                           .rearrange("p (k t) -> p k t", k=KI)
                           [:, g * 4:(g + 1) * 4,
                            (tt % 4) * P:(tt % 4 + 1) * P])
                    src = tp[:].rearrange("p (j t) -> p j t", j=4)
                    nc.scalar.activation(
                        dst, src, mybir.ActivationFunctionType.Copy)

            for half, xTh in ((0, xTa), (1, xTb)):
                for path in range(NP):
                    for st in range(ST):
                        r0 = (path * ST + st) * P
                        vtt = vt_pool.tile([P, KI * P], BF16, tag="vtst")
                        nc.sync.dma_start(
                            out=vtt[:], in_=vt_ap[r0:r0 + P, :])
                        psh = ps1.tile([P, 512], F32, tag="ps1")
                        for k in range(KI):
                            nc.tensor.matmul(
                                psh[:], vtt[:, k * P:(k + 1) * P],
                                xTh[:, k * 512:(k + 1) * 512],
                                start=(k == 0), stop=(k == KI - 1))
                        base = (path * ST + st) * TOK + half * 512
                        sccol = sc_sb[:, path * ST + st:path * ST + st + 1]
                        nc.scalar.activation(
                            hT[:, base:base + 512], psh[:],
                            mybir.ActivationFunctionType.Copy, scale=sccol)

    def phase2(nc, tc, hT):
        with tc.tile_pool(name="utq", bufs=24) as ut_pool, \
             tc.tile_pool(name="bbp", bufs=1) as bb_pool, \
             tc.tile_pool(name="ysb", bufs=4) as y_pool, \
             tc.tile_pool(name="ps2", bufs=8, space="PSUM") as ps2:
            bb_sb = bb_pool.tile([P, OUT_F], F32)
            nc.sync.dma_start(out=bb_sb[:], in_=bb_ap[:, :])
            for oq in range(OQ):
                uts = []
                for path in range(NP):
                    for k in range(ST):
                        r0 = ((path * OQ + oq) * ST + k) * P
                        u = ut_pool.tile([P, 2048], BF16, tag="utq")
                        nc.sync.dma_start(
                            out=u[:], in_=ut_ap[r0:r0 + P, :])
                        uts.append((path, k, u))
                for tt in range(TT):
                    pss = []
                    for _q in range(4):
                        psq = ps2.tile([P, 512], F32, tag="ps2")
                        pss.append(psq)
                    for i, (path, k, u) in enumerate(uts):
                        col = (path * ST + k) * TOK + tt * P
                        lhsT = hT[:, col:col + P]
                        for q in range(4):
                            nc.tensor.matmul(
                                pss[q][:], lhsT,
                                u[:, q * 512:(q + 1) * 512],
                                start=(i == 0), stop=(i == len(uts) - 1))
                    ysb = y_pool.tile([P, 2048], F32, tag="ysb")
                    for q in range(4):
                        nc.vector.tensor_add(
                            ysb[:, q * 512:(q + 1) * 512], pss[q][:],
                            bb_sb[:, oq * 2048 + q * 512:
                                  oq * 2048 + (q + 1) * 512])
                    nc.sync.dma_start(
                        out=y_ap[tt * P:(tt + 1) * P,
                                 oq * 2048:(oq + 1) * 2048],
                        in_=ysb[:])

    with tile.TileContext(nc) as tc:
        with tc.tile_pool(name="const", bufs=1) as const, \
             tc.tile_pool(name="ht", bufs=1) as ht_pool:
            ident = const.tile([P, P], BF16)
            make_identity(nc, ident[:])
            sc_sb = const.tile([P, NP * ST], F32)
            nc.sync.dma_start(out=sc_sb[:], in_=sc_ap[:, :])
            # hT[p, (path*8+st)*1024 + t] = scaled h.T, bf16
            hT = ht_pool.tile([P, NP * SPLIT * TT], BF16)
            for _rep in range(reps):
                phase1(nc, tc, ident, sc_sb, hT)
                phase2(nc, tc, hT)

    nc.compile()
    return nc


def _prep_host(x, V, U, v2, v1, u2, u1, V_R, U_R, v2_R, v1_R, u2_R, u1_R,
               bias):
    bf = ml_dtypes.bfloat16
    x2 = np.ascontiguousarray(np.asarray(x, dtype=np.float32)
                              .reshape(B * S, IN_F))

    def prep_vt(Vm, v2m):
        # VT[k, s] = sign(V[s, k]) * v2[k]  -> layout [st, p, k*128+j]
        VT = (np.sign(np.asarray(Vm, np.float32)).T
              * np.asarray(v2m, np.float32).reshape(IN_F, 1)).astype(bf)
        # VT [IN_F, SPLIT] -> [KI, P(p), ST, P(j)] -> [ST, P(p), KI, P(j)]
        return (VT.reshape(KI, P, ST, P).transpose(2, 1, 0, 3)
                .reshape(ST * P, KI * P))

    def prep_ut(Um, u1m):
        # UT[s, o] = sign(U[o, s]) * u1[o] -> layout [(oq, k, p), j]
        UT = (np.sign(np.asarray(Um, np.float32)).T
              * np.asarray(u1m, np.float32).reshape(1, OUT_F)).astype(bf)
        # UT [SPLIT, OUT_F] -> [ST, P, OQ, 2048] -> [OQ, ST, P, 2048]
        return (UT.reshape(ST, P, OQ, 2048).transpose(2, 0, 1, 3)
                .reshape(OQ * ST * P, 2048))

    vt_host = np.concatenate([prep_vt(V, v2), prep_vt(V_R, v2_R)], axis=0)
    ut_host = np.concatenate([prep_ut(U, u1), prep_ut(U_R, u1_R)], axis=0)

    sc_host = np.empty((P, NP * ST), np.float32)
    sc_host[:, 0:ST] = (np.asarray(v1, np.float32)
                        * np.asarray(u2, np.float32)).reshape(ST, P).T
    sc_host[:, ST:] = (np.asarray(v1_R, np.float32)
                       * np.asarray(u2_R, np.float32)).reshape(ST, P).T
    bb_host = np.tile(np.asarray(bias, np.float32).reshape(1, OUT_F), (P, 1))
    return x2, vt_host, ut_host, sc_host, bb_host


def kernel(x, V, U, v2, v1, u2, u1, V_R, U_R, v2_R, v1_R, u2_R, u1_R, bias):
    global last_exec_time_ns, last_results
    if 1 not in _CACHE:
        _CACHE[1] = _build()
    nc = _CACHE[1]

    x2, vt_host, ut_host, sc_host, bb_host = _prep_host(
        x, V, U, v2, v1, u2, u1, V_R, U_R, v2_R, v1_R, u2_R, u1_R, bias)

    in_maps = []
    for c in range(N_CORES):
        in_maps.append({
            "xs": np.ascontiguousarray(x2[c * TOK:(c + 1) * TOK]),
            "vt": vt_host,
            "ut": ut_host,
            "sc": sc_host,
            "bb": bb_host,
        })

    res = bass_utils.run_bass_kernel_spmd(
        nc, in_maps, core_ids=list(range(N_CORES)), trace=False)
    last_results = res
    out = np.concatenate([r["y"] for r in res.results], axis=0)
    return out.reshape(B, S, OUT_F).astype(np.float32)


def time_kernel(iters=8, reps=1, **inputs):
    """Time device execution: inputs pre-placed on device, min wall over iters."""
    import time as _time
    import jax
    from jax.sharding import Mesh, PartitionSpec, NamedSharding
    from jax.experimental.shard_map import shard_map
    from concourse import bass2jax

    if reps not in _CACHE:
        _CACHE[reps] = _build(reps)
    nc = _CACHE[reps]
    x2, vt_host, ut_host, sc_host, bb_host = _prep_host(**inputs)
    host = {"xs": x2.reshape(N_CORES, TOK, IN_F),
            "vt": vt_host, "ut": ut_host, "sc": sc_host, "bb": bb_host}

    bass2jax.install_neuronx_cc_hook()
    partition_name = (nc.partition_id_tensor.name
                      if nc.partition_id_tensor else None)
    in_names, out_names, out_avals, zero_outs = [], [], [], []
    for alloc in nc.m.functions[0].allocations:
        if not isinstance(alloc, mybir.MemoryLocationSet):
            continue
        name = alloc.memorylocations[0].name
        if alloc.kind == "ExternalInput":
            if name != partition_name:
                in_names.append(name)
        elif alloc.kind == "ExternalOutput":
            out_names.append(name)
            shape = tuple(alloc.tensor_shape)
            dtype = mybir.dt.np(alloc.dtype)
            out_avals.append(jax.core.ShapedArray(shape, dtype))
            zero_outs.append(np.zeros((N_CORES * shape[0], *shape[1:]), dtype))
    n_params = len(in_names)
    all_names = in_names + out_names
    if partition_name is not None:
        all_names = all_names + [partition_name]

    def _body(*args):
        operands = list(args)
        if partition_name is not None:
            operands.append(bass2jax.partition_id_tensor())
        outs = bass2jax._bass_exec_p.bind(
            *operands, out_avals=tuple(out_avals), in_names=tuple(all_names),
            out_names=tuple(out_names), lowering_input_output_aliases=(),
            sim_require_finite=True, sim_require_nnan=True, nc=nc)
        return tuple(outs)

    devices = jax.devices()[:N_CORES]
    mesh = Mesh(np.asarray(devices), ("core",))
    spec = NamedSharding(mesh, PartitionSpec("core"))
    donate = tuple(range(n_params, n_params + len(out_names)))
    sharded = jax.jit(
        shard_map(_body, mesh=mesh,
                  in_specs=(PartitionSpec("core"),) * (n_params + len(out_names)),
                  out_specs=(PartitionSpec("core"),) * len(out_names)),
        donate_argnums=donate, keep_unused=True)

    concat_in = []
    for name in in_names:
        h = host[name]
        if name == "xs":
            concat_in.append(np.ascontiguousarray(h.reshape(-1, IN_F)))
        else:
            concat_in.append(np.concatenate([h] * N_CORES, axis=0))
    dev_in = [jax.device_put(a, spec) for a in concat_in]
    jax.block_until_ready(dev_in)

    times = []
    out = None
    for _ in range(iters):
        dev_zero = [jax.device_put(z, spec) for z in zero_outs]
        jax.block_until_ready(dev_zero)
        t0 = _time.perf_counter()
        out = sharded(*dev_in, *dev_zero)
        jax.block_until_ready(out)
        times.append(_time.perf_counter() - t0)
    y = np.asarray(out[0]).reshape(B, S, OUT_F)
    return times, y


# revision 15
# speedup vs baseline: 114.0825x; 1.1206x over previous
"""Trainium2 Bass kernel for a double-path sign-quantized (ITQ) linear layer.

  y = ((x @ (sign(V).T * v2f)) * (v1*u2)) @ (sign(U).T scaled by u1)
      + same for _R path
      + bias

Sharding: data-parallel over tokens across 8 NeuronCores (8192 tokens -> 1024
per core). Weights are pre-quantized on host (sign() + scale folds + transpose
to contraction-major layout + bf16 cast); each core runs the full two-matmul
chain for its token slice. No collectives.

Device-side dataflow per core:
  phase 1: DMA-cast x slice f32->bf16, PE-transpose to xT [in, tok],
           mm1: hT[split, tok] = VT.T @ xT accumulated over 32 k-tiles,
           PSUM->SBUF copy applies (v1*u2) per-partition scale + bf16 cast.
  phase 2: mm2: y[tok, out] accumulates 16 (path, split-tile) rounds in PSUM,
           DVE adds broadcast bias, DMA out f32.
"""

import os
import sys

for _p in ("/opt/trn_rl_repo", "/root/.axon_site/_ro/trn_rl_repo"):
    if os.path.isdir(_p) and _p not in sys.path:
        sys.path.insert(0, _p)

import numpy as np
import ml_dtypes

import concourse.bass as bass
import concourse.mybir as mybir
import concourse.tile as tile
from concourse import bacc, bass_utils
from concourse.masks import make_identity

P = 128
IN_F, OUT_F, SPLIT = 4096, 4096, 1024
B, S = 2, 4096
N_CORES = 8
TOK = (B * S) // N_CORES        # 1024 tokens per core
KI = IN_F // P                  # 32 k-tiles for mm1
ST = SPLIT // P                 # 8 split tiles per path
TT = TOK // P                   # 8 token tiles
OQ = 2                          # out-feature quads of 2048
NP = 2                          # paths (main, residual)

F32 = mybir.dt.float32
BF16 = mybir.dt.bfloat16

_CACHE = {}
last_exec_time_ns = None
last_results = None


def _build(reps=1):
    nc = bacc.Bacc("TRN2", target_bir_lowering=False, debug=False,
                   num_devices=N_CORES)

    xs = nc.dram_tensor("xs", [TOK, IN_F], F32, kind="ExternalInput")
    # vt rows: (path*8+st)*128+p ; cols: k*128+j  == sign(V[st*128+j, k*128+p])*v2[k*128+p]
    vt = nc.dram_tensor("vt", [NP * ST * P, KI * P], BF16, kind="ExternalInput")
    # ut rows: ((path*2+oq)*8+k)*128+p ; cols j == sign(U[oq*2048+j, k*128+p])*u1[oq*2048+j]
    ut = nc.dram_tensor("ut", [NP * OQ * ST * P, 2048], BF16, kind="ExternalInput")
    sc = nc.dram_tensor("sc", [P, NP * ST], F32, kind="ExternalInput")
    bb = nc.dram_tensor("bb", [P, OUT_F], F32, kind="ExternalInput")
    y = nc.dram_tensor("y", [TOK, OUT_F], F32, kind="ExternalOutput")

    xs_ap, vt_ap, ut_ap, sc_ap, bb_ap, y_ap = (
        t.ap() for t in (xs, vt, ut, sc, bb, y))

    def phase1(nc, tc, ident, sc_sb, hT):
        with tc.tile_pool(name="xc", bufs=3) as xc_pool, \
             tc.tile_pool(name="xT", bufs=1) as xT_pool, \
             tc.tile_pool(name="vtst", bufs=3) as vt_pool, \
             tc.tile_pool(name="tpsum", bufs=2, space="PSUM") as tpsum, \
             tc.tile_pool(name="ps1", bufs=6, space="PSUM") as ps1:
            # xT split into token halves: the first half's matmul chains
            # start after only 4 x-tiles are cast+transposed.
            xTa = xT_pool.tile([P, KI * 512], BF16)
            xTb = xT_pool.tile([P, KI * 512], BF16)
            for tt in range(TT):
                xTh = xTa if tt < 4 else xTb
                xc = xc_pool.tile([P, IN_F], BF16, tag="xc")
                # SWDGE dma with f32->bf16 cast
                nc.gpsimd.dma_start(
                    out=xc[:], in_=xs_ap[tt * P:(tt + 1) * P, :])
                for g in range(KI // 4):
                    tp = tpsum.tile([P, 512], BF16, tag="tp")
                    for j in range(4):
                        k = g * 4 + j
                        nc.tensor.transpose(
                            tp[:, j * P:(j + 1) * P],
                            xc[:, k * P:(k + 1) * P], ident[:])
                    dst = (xTh[:]
                           .rearrange("p (k t) -> p k t", k=KI)
                           [:, g * 4:(g + 1) * 4,
                            (tt % 4) * P:(tt % 4 + 1) * P])
                    src = tp[:].rearrange("p (j t) -> p j t", j=4)
                    nc.scalar.activation(
                        dst, src, mybir.ActivationFunctionType.Copy)

            for half, xTh in ((0, xTa), (1, xTb)):
                for path in range(NP):
                    for st in range(ST):
                        r0 = (path * ST + st) * P
                        vtt = vt_pool.tile([P, KI * P], BF16, tag="vtst")
                        nc.sync.dma_start(
                            out=vtt[:], in_=vt_ap[r0:r0 + P, :])
                        psh = ps1.tile([P, 512], F32, tag="ps1")
                        for k in range(KI):
                            nc.tensor.matmul(
                                psh[:], vtt[:, k * P:(k + 1) * P],
                                xTh[:, k * 512:(k + 1) * 512],
                                start=(k == 0), stop=(k == KI - 1))
                        base = (path * ST + st) * TOK + half * 512
                        sccol = sc_sb[:, path * ST + st:path * ST + st + 1]
                        nc.scalar.activation(
                            hT[:, base:base + 512], psh[:],
                            mybir.ActivationFunctionType.Copy, scale=sccol)

    def phase2(nc, tc, hT):
        with tc.tile_pool(name="utq", bufs=24) as ut_pool, \
             tc.tile_pool(name="bbp", bufs=1) as bb_pool, \
             tc.tile_pool(name="ysb", bufs=4) as y_pool, \
             tc.tile_pool(name="ps2", bufs=8, space="PSUM") as ps2:
            bb_sb = bb_pool.tile([P, OUT_F], F32)
            nc.sync.dma_start(out=bb_sb[:], in_=bb_ap[:, :])
            for oq in range(OQ):
                uts = []
                for path in range(NP):
                    for k in range(ST):
                        r0 = ((path * OQ + oq) * ST + k) * P
                        u = ut_pool.tile([P, 2048], BF16, tag="utq")
                        nc.sync.dma_start(
                            out=u[:], in_=ut_ap[r0:r0 + P, :])
                        uts.append((path, k, u))
                for tt in range(TT):
                    pss = []
                    for _q in range(4):
                        psq = ps2.tile([P, 512], F32, tag="ps2")
                        pss.append(psq)
                    for i, (path, k, u) in enumerate(uts):
                        col = (path * ST + k) * TOK + tt * P
                        lhsT = hT[:, col:col + P]
                        for q in range(4):
                            nc.tensor.matmul(
                                pss[q][:], lhsT,
                                u[:, q * 512:(q + 1) * 512],
                                start=(i == 0), stop=(i == len(uts) - 1))
                    ysb = y_pool.tile([P, 2048], F32, tag="ysb")
                    for q in range(4):
                        nc.vector.tensor_add(
                            ysb[:, q * 512:(q + 1) * 512], pss[q][:],
                            bb_sb[:, oq * 2048 + q * 512:
                                  oq * 2048 + (q + 1) * 512])
                    nc.sync.dma_start(
                        out=y_ap[tt * P:(tt + 1) * P,
                                 oq * 2048:(oq + 1) * 2048],
                        in_=ysb[:])

    with tile.TileContext(nc) as tc:
        with tc.tile_pool(name="const", bufs=1) as const, \
             tc.tile_pool(name="ht", bufs=1) as ht_pool:
            ident = const.tile([P, P], BF16)
            make_identity(nc, ident[:])
            sc_sb = const.tile([P, NP * ST], F32)
            nc.sync.dma_start(out=sc_sb[:], in_=sc_ap[:, :])
            # hT[p, (path*8+st)*1024 + t] = scaled h.T, bf16
            hT = ht_pool.tile([P, NP * SPLIT * TT], BF16)
            for _rep in range(reps):
                phase1(nc, tc, ident, sc_sb, hT)
                phase2(nc, tc, hT)

    nc.compile()
    return nc


def _prep_host(x, V, U, v2, v1, u2, u1, V_R, U_R, v2_R, v1_R, u2_R, u1_R,
               bias):
    bf = ml_dtypes.bfloat16
    x2 = np.ascontiguousarray(np.asarray(x, dtype=np.float32)
                              .reshape(B * S, IN_F))

    def prep_vt(Vm, v2m):
        # VT[k, s] = sign(V[s, k]) * v2[k]  -> layout [st, p, k*128+j]
        VT = (np.sign(np.asarray(Vm, np.float32)).T
              * np.asarray(v2m, np.float32).reshape(IN_F, 1)).astype(bf)
        # VT [IN_F, SPLIT] -> [KI, P(p), ST, P(j)] -> [ST, P(p), KI, P(j)]
        return (VT.reshape(KI, P, ST, P).transpose(2, 1, 0, 3)
                .reshape(ST * P, KI * P))

    def prep_ut(Um, u1m):
        # UT[s, o] = sign(U[o, s]) * u1[o] -> layout [(oq, k, p), j]
        UT = (np.sign(np.asarray(Um, np.float32)).T
              * np.asarray(u1m, np.float32).reshape(1, OUT_F)).astype(bf)
        # UT [SPLIT, OUT_F] -> [ST, P, OQ, 2048] -> [OQ, ST, P, 2048]
        return (UT.reshape(ST, P, OQ, 2048).transpose(2, 0, 1, 3)
                .reshape(OQ * ST * P, 2048))

    vt_host = np.concatenate([prep_vt(V, v2), prep_vt(V_R, v2_R)], axis=0)
    ut_host = np.concatenate([prep_ut(U, u1), prep_ut(U_R, u1_R)], axis=0)

    sc_host = np.empty((P, NP * ST), np.float32)
    sc_host[:, 0:ST] = (np.asarray(v1, np.float32)
                        * np.asarray(u2, np.float32)).reshape(ST, P).T
    sc_host[:, ST:] = (np.asarray(v1_R, np.float32)
                       * np.asarray(u2_R, np.float32)).reshape(ST, P).T
    bb_host = np.tile(np.asarray(bias, np.float32).reshape(1, OUT_F), (P, 1))
    return x2, vt_host, ut_host, sc_host, bb_host


def kernel(x, V, U, v2, v1, u2, u1, V_R, U_R, v2_R, v1_R, u2_R, u1_R, bias):
    global last_exec_time_ns, last_results
    if 1 not in _CACHE:
        _CACHE[1] = _build()
    nc = _CACHE[1]

    x2, vt_host, ut_host, sc_host, bb_host = _prep_host(
        x, V, U, v2, v1, u2, u1, V_R, U_R, v2_R, v1_R, u2_R, u1_R, bias)

    in_maps = []
    for c in range(N_CORES):
        in_maps.append({
            "xs": np.ascontiguousarray(x2[c * TOK:(c + 1) * TOK]),
            "vt": vt_host,
            "ut": ut_host,
            "sc": sc_host,
            "bb": bb_host,
        })

    res = bass_utils.run_bass_kernel_spmd(
        nc, in_maps, core_ids=list(range(N_CORES)), trace=False)
    last_results = res
    out = np.concatenate([r["y"] for r in res.results], axis=0)
    return out.reshape(B, S, OUT_F).astype(np.float32)


def time_kernel(iters=8, reps=1, **inputs):
    """Time device execution: inputs pre-placed on device, min wall over iters."""
    import time as _time
    import jax
    from jax.sharding import Mesh, PartitionSpec, NamedSharding
    from jax.experimental.shard_map import shard_map
    from concourse import bass2jax

    if reps not in _CACHE:
        _CACHE[reps] = _build(reps)
    nc = _CACHE[reps]
    x2, vt_host, ut_host, sc_host, bb_host = _prep_host(**inputs)
    host = {"xs": x2.reshape(N_CORES, TOK, IN_F),
            "vt": vt_host, "ut": ut_host, "sc": sc_host, "bb": bb_host}

    bass2jax.install_neuronx_cc_hook()
    partition_name = (nc.partition_id_tensor.name
                      if nc.partition_id_tensor else None)
    in_names, out_names, out_avals, zero_outs = [], [], [], []
    for alloc in nc.m.functions[0].allocations:
        if not isinstance(alloc, mybir.MemoryLocationSet):
            continue
        name = alloc.memorylocations[0].name
        if alloc.kind == "ExternalInput":
            if name != partition_name:
                in_names.append(name)
        elif alloc.kind == "ExternalOutput":
            out_names.append(name)
            shape = tuple(alloc.tensor_shape)
            dtype = mybir.dt.np(alloc.dtype)
            out_avals.append(jax.core.ShapedArray(shape, dtype))
            zero_outs.append(np.zeros((N_CORES * shape[0], *shape[1:]), dtype))
    n_params = len(in_names)
    all_names = in_names + out_names
    if partition_name is not None:
        all_names = all_names + [partition_name]

    def _body(*args):
        operands = list(args)
        if partition_name is not None:
            operands.append(bass2jax.partition_id_tensor())
        outs = bass2jax._bass_exec_p.bind(
            *operands, out_avals=tuple(out_avals), in_names=tuple(all_names),
            out_names=tuple(out_names), lowering_input_output_aliases=(),
            sim_require_finite=True, sim_require_nnan=True, nc=nc)
        return tuple(outs)

    devices = jax.devices()[:N_CORES]
    mesh = Mesh(np.asarray(devices), ("core",))
    spec = NamedSharding(mesh, PartitionSpec("core"))
    donate = tuple(range(n_params, n_params + len(out_names)))
    sharded = jax.jit(
        shard_map(_body, mesh=mesh,
                  in_specs=(PartitionSpec("core"),) * (n_params + len(out_names)),
                  out_specs=(PartitionSpec("core"),) * len(out_names)),
        donate_argnums=donate, keep_unused=True)

    concat_in = []
    for name in in_names:
        h = host[name]
        if name == "xs":
            concat_in.append(np.ascontiguousarray(h.reshape(-1, IN_F)))
        else:
            concat_in.append(np.concatenate([h] * N_CORES, axis=0))
    dev_in = [jax.device_put(a, spec) for a in concat_in]
    jax.block_until_ready(dev_in)

    times = []
    out = None
    for _ in range(iters):
        dev_zero = [jax.device_put(z, spec) for z in zero_outs]
        jax.block_until_ready(dev_zero)
        t0 = _time.perf_counter()
        out = sharded(*dev_in, *dev_zero)
        jax.block_until_ready(out)
        times.append(_time.perf_counter() - t0)
    y = np.asarray(out[0]).reshape(B, S, OUT_F)
    return times, y
